# revision 24
# baseline (speedup 1.0000x reference)
import re
import sys

sys.path.insert(0, "/opt/trn_rl_repo")

import ml_dtypes
import numpy as np

from concourse import bass, mybir
from concourse.bass_utils import run_bass_kernel_spmd
from concourse.tile import TileContext
from concourse.vector_clock import ScopedClock, VectorClock


def _drain_and_barrier_split_waits(self, tick_clock, wait_clock):
    # Walrus codegen caps sync waits per CTRL instruction at 2; the stock
    # exit drain carries one wait per busy proc and fails to compile. Emit
    # one single-wait SP nop per proc ahead of the drain instead.
    ticks = [int(v) for v in re.findall(r"\d+", str(tick_clock.global_clock))]
    for p, t in enumerate(ticks):
        if t > 0:
            partial = VectorClock()
            partial.require_at_least(p, t)
            nop = self.nc.sync.nop(nofuse=True)
            wait_clock.add_sem_waits(nop.ins, ScopedClock({None: partial}))
    self.nc.sync.drain()
    self.nc.all_engine_barrier()
    assert self.sems is not None
    popped = self.nc._tile_sem_poison_stack.pop()
    assert popped is self._sem_poison
    self.nc.clear_and_free_semaphores(list(self.sems.allocated().values()))
    self.nc.all_engine_barrier()


TileContext._drain_and_barrier = _drain_and_barrier_split_waits

B, L = 131072, 256
NCORES = 8
RPC = B // NCORES           # rows per core = 16384
P = 128                     # partitions
NT = RPC // P               # segments per partition = 128
NSEP = 4                    # separator columns per segment
SEG_W = L + NSEP            # 260
NCH = 8                     # chunks per core
CSEG = NT // NCH            # segments per chunk = 16
FC = CSEG * SEG_W           # free elems per chunk = 4160
W = 10                      # positions folded per psum column (260 = 26*10)
NMM = SEG_W // W            # matmuls per stream-chunk = 26
SENT = 1024.0               # separator value (bf16-exact, > any id)
INIT1 = -1024.0             # forward-scan initial
INITR = -896.0              # reverse-scan initial
DT = mybir.dt.bfloat16
F32 = mybir.dt.float32
Alu = mybir.AluOpType
Act = mybir.ActivationFunctionType

NSTREAM = 5                 # m, sgn, rsgn, e, me

_cache = {}


def build():
    nc = bass.Bass()
    x = nc.declare_dram_parameter("x", [NCH * P, FC], DT, isOutput=False)
    out = nc.declare_dram_parameter("out", [RPC, 4], F32, isOutput=True)
    out_re = out.rearrange("(p n) k -> p (n k)", p=P)   # [128, NT*4]

    with TileContext(nc) as tc:
        with (
            tc.tile_pool(name="persist", bufs=1) as pp,
            tc.tile_pool(name="work", bufs=2) as wp,
            tc.psum_pool(name="ps", bufs=1) as psp,
        ):
            # per-cell partial sums are <= 26, bf16-exact; split by writing
            # engine so each final reduce waits on a single semaphore
            stageD = pp.tile([P, 3 * NCH * CSEG * W], DT)
            stageA = pp.tile([P, 2 * NCH * CSEG * W], DT)
            stats = pp.tile([P, NSTREAM * NT], F32)
            feat = pp.tile([P, NT * 4], F32)

            # identity (stationary weights) built on-chip so matmuls depend
            # on DVE sems only (walrus MM struct allows a single sync wait)
            iota_row = pp.tile([P, P], mybir.dt.int32)
            iota_col = pp.tile([P, 1], F32)
            identt = pp.tile([P, P], DT)
            nc.gpsimd.iota(iota_row, [[1, P]], channel_multiplier=0)
            nc.gpsimd.iota(iota_col, [[0, 1]], channel_multiplier=1,
                           allow_small_or_imprecise_dtypes=True)
            nc.vector.tensor_scalar(identt, iota_row, iota_col, None, Alu.is_equal)

            # one xc buffer per chunk, all DMAs issued up front: no WAR/WAW
            # waits, and the DMA engine runs well ahead of compute
            xcs = []
            for c in range(NCH):
                xc = wp.tile([P, FC], DT, tag="xc", bufs=NCH)
                nc.gpsimd.dma_start(out=xc, in_=x[c * P:(c + 1) * P, :])
                xcs.append(xc)

            for c in range(NCH):
                xc = xcs[c]
                m = wp.tile([P, FC], DT, tag="m", bufs=4)
                v1buf = wp.tile([P, FC + 2], DT, tag="v1buf")
                f1buf = wp.tile([P, FC + 2], DT, tag="f1buf")
                sgn = wp.tile([P, FC], DT, tag="sgn")
                rsgn = wp.tile([P, FC], DT, tag="rsgn")
                me = wp.tile([P, FC], DT, tag="me")

                # m = [x == 0] on DVE (keeps xc single-engine-read so the next
                # chunk's DMA WAR carries one semaphore; scans then wait on DVE
                # program order only)
                nc.vector.tensor_scalar(m, xc, 0.5, None, Alu.is_lt)
                nc.vector.memset(v1buf[:, 0:1], INIT1)
                nc.vector.memset(f1buf[:, FC + 1:FC + 2], INITR)

                # v1buf[1+t] = last nonzero at-or-before t  (v1[t] = v1buf[t])
                nc.vector.tensor_tensor_scan(
                    out=v1buf[:, 1:FC + 1], data0=m, data1=xc,
                    initial=INIT1, op0=Alu.mult, op1=Alu.add,
                )
                v1 = v1buf[:, 0:FC]
                # f1buf[1+t] = next nonzero at-or-after t (f1[t] = f1buf[t+2])
                nc.vector.tensor_tensor_scan(
                    out=f1buf[:, 1:FC + 1][:, ::-1], data0=m[:, ::-1],
                    data1=xc[:, ::-1],
                    initial=INITR, op0=Alu.mult, op1=Alu.add,
                )
                f1 = f1buf[:, 2:FC + 2]

                # d1 = x - v1 in place over xc; sign pair on ScalarE
                d1 = xc
                nc.vector.tensor_tensor(d1, xc, v1, Alu.subtract)
                nc.scalar.activation(sgn, d1, Act.Sign)
                nc.scalar.activation(rsgn, sgn, Act.Relu)
                # e in place over v1 (same-index elementwise)
                e = v1
                nc.vector.tensor_tensor(e, v1, f1, Alu.is_equal)
                nc.vector.tensor_tensor(me, m, e, Alu.mult)

                # per-segment sums via identity-matmul PSUM accumulation.
                # e/me/m are DVE-written: fold W and land in stats via a DVE
                # tensor_reduce straight from PSUM (single DVE-sem waits all
                # around). sgn/rsgn are ACT-written: ACT copies their psums to
                # stage2 so their matmuls' WAR stays on the ACT sem.
                for s_idx, s in enumerate((e, me, m, sgn, rsgn)):
                    if s_idx < 3:
                        ps = psp.tile([P, CSEG * W], F32, tag="ps", bufs=3)
                    else:
                        ps = psp.tile([P, CSEG * W], F32, tag="ps2", bufs=2)
                    s3 = s.rearrange("p (n w) -> p n w", w=SEG_W)
                    for cc in range(NMM):
                        nc.tensor.matmul(
                            out=ps, lhsT=identt,
                            rhs=s3[:, :, W * cc:W * cc + W],
                            start=(cc == 0), stop=(cc == NMM - 1),
                        )
                    if s_idx < 3:
                        dst = stageD[:, (s_idx * NCH + c) * CSEG * W:
                                     (s_idx * NCH + c + 1) * CSEG * W]
                        nc.vector.tensor_copy(dst, ps)
                    else:
                        dst = stageA[:, ((s_idx - 3) * NCH + c) * CSEG * W:
                                     ((s_idx - 3) * NCH + c + 1) * CSEG * W]
                        nc.scalar.copy(dst, ps)  # ACT streams stay on ACT sem

            stD = stageD.rearrange("p (q w) -> p q w", w=W)
            nc.vector.tensor_reduce(stats[:, 0:3 * NT], stD,
                                    mybir.AxisListType.X, Alu.add)
            stA = stageA.rearrange("p (q w) -> p q w", w=W)
            nc.vector.tensor_reduce(stats[:, 3 * NT:5 * NT], stA,
                                    mybir.AxisListType.X, Alu.add)

            def st(k):
                return stats[:, k * NT:(k + 1) * NT]

            E, ME, zc, D, pos = st(0), st(1), st(2), st(3), st(4)

            ep = pp
            n = ep.tile([P, NT], F32)
            nc.vector.tensor_scalar(n, zc, -1.0, float(L), Alu.mult, Alu.add)
            # rep = 257 - 2*pos + D ; inc = pos - 1 ; dec = pos - D - zc - 1
            inc = ep.tile([P, NT], F32)
            nc.vector.tensor_scalar(inc, pos, -1.0, None, Alu.add)
            t257 = ep.tile([P, NT], F32)
            nc.vector.tensor_scalar(t257, pos, -2.0, 257.0, Alu.mult, Alu.add)
            rep = ep.tile([P, NT], F32)
            nc.vector.tensor_tensor(rep, t257, D, Alu.add)
            u1 = ep.tile([P, NT], F32)
            nc.vector.tensor_tensor(u1, pos, D, Alu.subtract)
            u2 = ep.tile([P, NT], F32)
            nc.vector.tensor_tensor(u2, u1, zc, Alu.subtract)
            dec = ep.tile([P, NT], F32)
            nc.vector.tensor_scalar(dec, u2, -1.0, None, Alu.add)
            em2 = ep.tile([P, NT], F32)
            nc.vector.tensor_scalar(em2, E, -2.0, None, Alu.add)
            per = ep.tile([P, NT], F32)
            nc.vector.tensor_tensor(per, em2, ME, Alu.subtract)

            d1v = ep.tile([P, NT], F32)
            nc.vector.tensor_scalar(d1v, n, -1.0, 1.0, Alu.add, Alu.max)
            r1 = ep.tile([P, NT], F32)
            nc.vector.reciprocal(r1, d1v)
            d2v = ep.tile([P, NT], F32)
            nc.vector.tensor_scalar(d2v, n, -2.0, 1.0, Alu.add, Alu.max)
            r2 = ep.tile([P, NT], F32)
            nc.vector.reciprocal(r2, d2v)
            mask2 = ep.tile([P, NT], F32)
            nc.vector.tensor_scalar(mask2, n, 1.5, None, Alu.is_gt)
            mask4 = ep.tile([P, NT], F32)
            nc.vector.tensor_scalar(mask4, n, 3.5, None, Alu.is_gt)
            r1m = ep.tile([P, NT], F32)
            nc.vector.tensor_tensor(r1m, r1, mask2, Alu.mult)
            r2m = ep.tile([P, NT], F32)
            nc.vector.tensor_tensor(r2m, r2, mask4, Alu.mult)

            feat3 = feat.rearrange("p (n k) -> p n k", k=4)
            nc.vector.tensor_tensor(feat3[:, :, 0:1], rep, r1m, Alu.mult)
            nc.vector.tensor_tensor(feat3[:, :, 1:2], inc, r1m, Alu.mult)
            nc.vector.tensor_tensor(feat3[:, :, 2:3], dec, r1m, Alu.mult)
            nc.vector.tensor_tensor(feat3[:, :, 3:4], per, r2m, Alu.mult)

            # HWDGE (sync-engine) DMA: its completion sem is separate from the
            # 8-deep SWDGE sem pool, so no sem-reuse wait on input DMA 0
            nc.sync.dma_start(out=out_re, in_=feat)

    return nc


def _prep_core(x_core):
    """[RPC, 256] int32 -> [NCH*P, FC] bf16: pad 4 SENT sep cols per row,
    then reorder rows p*NT+j so each chunk (16 segments x all partitions)
    is one contiguous [128, FC] DMA block."""
    xpad = np.empty((RPC, SEG_W), dtype=ml_dtypes.bfloat16)
    xpad[:, :NSEP] = ml_dtypes.bfloat16(SENT)
    xpad[:, NSEP:] = x_core.astype(ml_dtypes.bfloat16)
    arr = xpad.reshape(P, NCH, CSEG, SEG_W).transpose(1, 0, 2, 3)
    return np.ascontiguousarray(arr.reshape(NCH * P, FC))


def kernel(x: np.ndarray) -> np.ndarray:
    if "nc" not in _cache:
        _cache["nc"] = build()
    nc = _cache["nc"]
    x = np.ascontiguousarray(np.asarray(x, dtype=np.int32))
    shards = x.reshape(NCORES, RPC, L)
    in_maps = [{"x": _prep_core(shards[i])} for i in range(NCORES)]
    res = run_bass_kernel_spmd(nc, in_maps, list(range(NCORES)))
    outs = [res.results[i]["out"] for i in range(NCORES)]
    return np.concatenate(outs, axis=0).astype(np.float32)


# revision 27
# speedup vs baseline: 1.0957x; 1.0957x over previous
import re
import sys

sys.path.insert(0, "/opt/trn_rl_repo")

import ml_dtypes
import numpy as np

from concourse import bass, mybir
from concourse.bass_utils import run_bass_kernel_spmd
from concourse.tile import TileContext
from concourse.vector_clock import ScopedClock, VectorClock


def _drain_and_barrier_split_waits(self, tick_clock, wait_clock):
    # Walrus codegen caps sync waits per CTRL instruction at 2; the stock
    # exit drain carries one wait per busy proc and fails to compile. Emit
    # one single-wait SP nop per proc ahead of the drain instead.
    ticks = [int(v) for v in re.findall(r"\d+", str(tick_clock.global_clock))]
    for p, t in enumerate(ticks):
        if t > 0:
            partial = VectorClock()
            partial.require_at_least(p, t)
            nop = self.nc.sync.nop(nofuse=True)
            wait_clock.add_sem_waits(nop.ins, ScopedClock({None: partial}))
    self.nc.sync.drain()
    self.nc.all_engine_barrier()
    assert self.sems is not None
    popped = self.nc._tile_sem_poison_stack.pop()
    assert popped is self._sem_poison
    self.nc.clear_and_free_semaphores(list(self.sems.allocated().values()))
    self.nc.all_engine_barrier()


TileContext._drain_and_barrier = _drain_and_barrier_split_waits

B, L = 131072, 256
NCORES = 8
RPC = B // NCORES           # rows per core = 16384
P = 128                     # partitions
NT = RPC // P               # segments per partition = 128
NSEP = 4                    # separator columns per segment
SEG_W = L + NSEP            # 260
NCH = 8                     # chunks per core
CSEG = NT // NCH            # segments per chunk = 16
FC = CSEG * SEG_W           # free elems per chunk = 4160
W = 10                      # positions folded per psum column (260 = 26*10)
NMM = SEG_W // W            # matmuls per stream-chunk = 26
SENT = 1024.0               # separator value (bf16-exact, > any id)
INIT1 = -1024.0             # forward-scan initial
INITR = -896.0              # reverse-scan initial
DT = mybir.dt.bfloat16
F32 = mybir.dt.float32
Alu = mybir.AluOpType
Act = mybir.ActivationFunctionType

NSTREAM = 5                 # m, sgn, rsgn, e, me

_cache = {}


def build():
    nc = bass.Bass()
    x = nc.declare_dram_parameter("x", [NCH * P, FC], DT, isOutput=False)
    out = nc.declare_dram_parameter("out", [RPC, 4], F32, isOutput=True)
    out_re = out.rearrange("(p n) k -> p (n k)", p=P)   # [128, NT*4]

    with TileContext(nc) as tc:
        with (
            tc.tile_pool(name="persist", bufs=1) as pp,
            tc.tile_pool(name="work", bufs=2) as wp,
            tc.psum_pool(name="ps", bufs=1) as psp,
        ):
            # per-cell partial sums are <= 26, bf16-exact; split by writing
            # engine so each final reduce waits on a single semaphore
            stageD = pp.tile([P, 3 * NCH * CSEG * W], DT)
            stageA = pp.tile([P, 2 * NCH * CSEG * W], DT)
            stats = pp.tile([P, NSTREAM * NT], F32)
            feat = pp.tile([P, NT * 4], F32)

            # identity (stationary weights) built on-chip so matmuls depend
            # on DVE sems only (walrus MM struct allows a single sync wait)
            iota_row = pp.tile([P, P], mybir.dt.int32)
            iota_col = pp.tile([P, 1], F32)
            identt = pp.tile([P, P], DT)
            nc.gpsimd.iota(iota_row, [[1, P]], channel_multiplier=0)
            nc.gpsimd.iota(iota_col, [[0, 1]], channel_multiplier=1,
                           allow_small_or_imprecise_dtypes=True)
            nc.vector.tensor_scalar(identt, iota_row, iota_col, None, Alu.is_equal)
            negident = pp.tile([P, P], DT)
            nc.vector.tensor_scalar(negident, identt, -1.0, None, Alu.mult)
            warmps = psp.tile([P, 16], F32)
            # warm-up matmul: advances PE's DVE clock past identt/negident so
            # later matmuls need no extra identity wait
            nc.tensor.matmul(out=warmps, lhsT=identt, rhs=negident[:, 0:16],
                             start=True, stop=True)

            # one xc buffer per chunk, all DMAs issued up front: no WAR/WAW
            # waits, and the DMA engine runs well ahead of compute
            xcs = []
            for c in range(NCH):
                xc = wp.tile([P, FC], DT, tag="xc", bufs=NCH)
                nc.gpsimd.dma_start(out=xc, in_=x[c * P:(c + 1) * P, :])
                xcs.append(xc)

            for c in range(NCH):
                xc = xcs[c]
                m = wp.tile([P, FC], DT, tag="m", bufs=4)
                v1buf = wp.tile([P, FC + 2], DT, tag="v1buf")
                f1buf = wp.tile([P, FC + 2], DT, tag="f1buf")
                sgn = wp.tile([P, FC], DT, tag="sgn")
                rsgn = wp.tile([P, FC], DT, tag="rsgn")
                me = wp.tile([P, FC], DT, tag="me")

                # m = [x == 0] on DVE (keeps xc single-engine-read so the next
                # chunk's DMA WAR carries one semaphore; scans then wait on DVE
                # program order only)
                nc.vector.tensor_scalar(m, xc, 0.5, None, Alu.is_lt)
                nc.vector.memset(v1buf[:, 0:1], INIT1)
                nc.vector.memset(f1buf[:, FC + 1:FC + 2], INITR)

                # v1buf[1+t] = last nonzero at-or-before t  (v1[t] = v1buf[t])
                nc.vector.tensor_tensor_scan(
                    out=v1buf[:, 1:FC + 1], data0=m, data1=xc,
                    initial=INIT1, op0=Alu.mult, op1=Alu.add,
                )
                v1 = v1buf[:, 0:FC]
                # f1buf[1+t] = next nonzero at-or-after t (f1[t] = f1buf[t+2])
                nc.vector.tensor_tensor_scan(
                    out=f1buf[:, 1:FC + 1][:, ::-1], data0=m[:, ::-1],
                    data1=xc[:, ::-1],
                    initial=INITR, op0=Alu.mult, op1=Alu.add,
                )
                f1 = f1buf[:, 2:FC + 2]

                # PE toucher: one tiny matmul waiting on this chunk's DMA so
                # the d1 matmuls below carry a single (WAR) wait each
                nc.tensor.matmul(out=warmps, lhsT=identt, rhs=xc[:, 0:16],
                                 start=True, stop=True)
                # d1 = x - v1 computed on the PE: psum piece = I@x + (-I)@v1;
                # ScalarE reads Sign straight from PSUM
                NP_PIECE = 416
                for pi in range(FC // NP_PIECE):
                    lo, hi = pi * NP_PIECE, (pi + 1) * NP_PIECE
                    psd = psp.tile([P, NP_PIECE], F32, tag="psd", bufs=3,
                                   name=f"psd_{c}_{pi}")
                    nc.tensor.matmul(out=psd, lhsT=identt, rhs=xc[:, lo:hi],
                                     start=True, stop=False)
                    nc.tensor.matmul(out=psd, lhsT=negident, rhs=v1[:, lo:hi],
                                     start=False, stop=True)
                    nc.scalar.activation(sgn[:, lo:hi], psd, Act.Sign)
                nc.scalar.activation(rsgn, sgn, Act.Relu)
                # e in place over f1 (same-index elementwise; v1 must stay
                # intact for the PE's d1 matmuls)
                e = f1
                nc.vector.tensor_tensor(e, v1, f1, Alu.is_equal)
                nc.vector.tensor_tensor(me, m, e, Alu.mult)

                # per-segment sums via identity-matmul PSUM accumulation.
                # e/me/m are DVE-written: fold W and land in stats via a DVE
                # tensor_reduce straight from PSUM (single DVE-sem waits all
                # around). sgn/rsgn are ACT-written: ACT copies their psums to
                # stage2 so their matmuls' WAR stays on the ACT sem.
                for s_idx, s in enumerate((e, me, m, sgn, rsgn)):
                    if s_idx < 3:
                        ps = psp.tile([P, CSEG * W], F32, tag="ps", bufs=2)
                    else:
                        ps = psp.tile([P, CSEG * W], F32, tag="ps2", bufs=2)
                    s3 = s.rearrange("p (n w) -> p n w", w=SEG_W)
                    for cc in range(NMM):
                        nc.tensor.matmul(
                            out=ps, lhsT=identt,
                            rhs=s3[:, :, W * cc:W * cc + W],
                            start=(cc == 0), stop=(cc == NMM - 1),
                        )
                    if s_idx < 3:
                        dst = stageD[:, (s_idx * NCH + c) * CSEG * W:
                                     (s_idx * NCH + c + 1) * CSEG * W]
                        nc.vector.tensor_copy(dst, ps)
                    else:
                        dst = stageA[:, ((s_idx - 3) * NCH + c) * CSEG * W:
                                     ((s_idx - 3) * NCH + c + 1) * CSEG * W]
                        nc.scalar.copy(dst, ps)  # ACT streams stay on ACT sem

            stD = stageD.rearrange("p (q w) -> p q w", w=W)
            nc.vector.tensor_reduce(stats[:, 0:3 * NT], stD,
                                    mybir.AxisListType.X, Alu.add)
            stA = stageA.rearrange("p (q w) -> p q w", w=W)
            nc.vector.tensor_reduce(stats[:, 3 * NT:5 * NT], stA,
                                    mybir.AxisListType.X, Alu.add)

            def st(k):
                return stats[:, k * NT:(k + 1) * NT]

            E, ME, zc, D, pos = st(0), st(1), st(2), st(3), st(4)

            ep = pp
            n = ep.tile([P, NT], F32)
            nc.vector.tensor_scalar(n, zc, -1.0, float(L), Alu.mult, Alu.add)
            # rep = 257 - 2*pos + D ; inc = pos - 1 ; dec = pos - D - zc - 1
            inc = ep.tile([P, NT], F32)
            nc.vector.tensor_scalar(inc, pos, -1.0, None, Alu.add)
            t257 = ep.tile([P, NT], F32)
            nc.vector.tensor_scalar(t257, pos, -2.0, 257.0, Alu.mult, Alu.add)
            rep = ep.tile([P, NT], F32)
            nc.vector.tensor_tensor(rep, t257, D, Alu.add)
            u1 = ep.tile([P, NT], F32)
            nc.vector.tensor_tensor(u1, pos, D, Alu.subtract)
            u2 = ep.tile([P, NT], F32)
            nc.vector.tensor_tensor(u2, u1, zc, Alu.subtract)
            dec = ep.tile([P, NT], F32)
            nc.vector.tensor_scalar(dec, u2, -1.0, None, Alu.add)
            em2 = ep.tile([P, NT], F32)
            nc.vector.tensor_scalar(em2, E, -2.0, None, Alu.add)
            per = ep.tile([P, NT], F32)
            nc.vector.tensor_tensor(per, em2, ME, Alu.subtract)

            d1v = ep.tile([P, NT], F32)
            nc.vector.tensor_scalar(d1v, n, -1.0, 1.0, Alu.add, Alu.max)
            r1 = ep.tile([P, NT], F32)
            nc.vector.reciprocal(r1, d1v)
            d2v = ep.tile([P, NT], F32)
            nc.vector.tensor_scalar(d2v, n, -2.0, 1.0, Alu.add, Alu.max)
            r2 = ep.tile([P, NT], F32)
            nc.vector.reciprocal(r2, d2v)
            mask2 = ep.tile([P, NT], F32)
            nc.vector.tensor_scalar(mask2, n, 1.5, None, Alu.is_gt)
            mask4 = ep.tile([P, NT], F32)
            nc.vector.tensor_scalar(mask4, n, 3.5, None, Alu.is_gt)
            r1m = ep.tile([P, NT], F32)
            nc.vector.tensor_tensor(r1m, r1, mask2, Alu.mult)
            r2m = ep.tile([P, NT], F32)
            nc.vector.tensor_tensor(r2m, r2, mask4, Alu.mult)

            feat3 = feat.rearrange("p (n k) -> p n k", k=4)
            nc.vector.tensor_tensor(feat3[:, :, 0:1], rep, r1m, Alu.mult)
            nc.vector.tensor_tensor(feat3[:, :, 1:2], inc, r1m, Alu.mult)
            nc.vector.tensor_tensor(feat3[:, :, 2:3], dec, r1m, Alu.mult)
            nc.vector.tensor_tensor(feat3[:, :, 3:4], per, r2m, Alu.mult)

            # HWDGE (sync-engine) DMA: its completion sem is separate from the
            # 8-deep SWDGE sem pool, so no sem-reuse wait on input DMA 0
            nc.sync.dma_start(out=out_re, in_=feat)

    return nc


def _prep_core(x_core):
    """[RPC, 256] int32 -> [NCH*P, FC] bf16: pad 4 SENT sep cols per row,
    then reorder rows p*NT+j so each chunk (16 segments x all partitions)
    is one contiguous [128, FC] DMA block."""
    xpad = np.empty((RPC, SEG_W), dtype=ml_dtypes.bfloat16)
    xpad[:, :NSEP] = ml_dtypes.bfloat16(SENT)
    xpad[:, NSEP:] = x_core.astype(ml_dtypes.bfloat16)
    arr = xpad.reshape(P, NCH, CSEG, SEG_W).transpose(1, 0, 2, 3)
    return np.ascontiguousarray(arr.reshape(NCH * P, FC))


def kernel(x: np.ndarray) -> np.ndarray:
    if "nc" not in _cache:
        _cache["nc"] = build()
    nc = _cache["nc"]
    x = np.ascontiguousarray(np.asarray(x, dtype=np.int32))
    shards = x.reshape(NCORES, RPC, L)
    in_maps = [{"x": _prep_core(shards[i])} for i in range(NCORES)]
    res = run_bass_kernel_spmd(nc, in_maps, list(range(NCORES)))
    outs = [res.results[i]["out"] for i in range(NCORES)]
    return np.concatenate(outs, axis=0).astype(np.float32)


# revision 29
# speedup vs baseline: 1.3255x; 1.2097x over previous
import re
import sys

sys.path.insert(0, "/opt/trn_rl_repo")

import ml_dtypes
import numpy as np

from concourse import bass, mybir
from concourse.bass_utils import run_bass_kernel_spmd
from concourse.tile import TileContext
from concourse.vector_clock import ScopedClock, VectorClock


def _drain_and_barrier_split_waits(self, tick_clock, wait_clock):
    # Walrus codegen caps sync waits per CTRL instruction at 2; the stock
    # exit drain carries one wait per busy proc and fails to compile. Emit
    # one single-wait SP nop per proc ahead of the drain instead.
    ticks = [int(v) for v in re.findall(r"\d+", str(tick_clock.global_clock))]
    for p, t in enumerate(ticks):
        if t > 0:
            partial = VectorClock()
            partial.require_at_least(p, t)
            nop = self.nc.sync.nop(nofuse=True)
            wait_clock.add_sem_waits(nop.ins, ScopedClock({None: partial}))
    self.nc.sync.drain()
    self.nc.all_engine_barrier()
    assert self.sems is not None
    popped = self.nc._tile_sem_poison_stack.pop()
    assert popped is self._sem_poison
    self.nc.clear_and_free_semaphores(list(self.sems.allocated().values()))
    self.nc.all_engine_barrier()


TileContext._drain_and_barrier = _drain_and_barrier_split_waits

B, L = 131072, 256
NCORES = 8
RPC = B // NCORES           # rows per core = 16384
P = 128                     # partitions
NT = RPC // P               # segments per partition = 128
NSEP = 4                    # separator columns per segment
SEG_W = L + NSEP            # 260
NCH = 8                     # chunks per core
CSEG = NT // NCH            # segments per chunk = 16
FC = CSEG * SEG_W           # free elems per chunk = 4160
W = 10                      # positions folded per psum column (260 = 26*10)
NMM = SEG_W // W            # matmuls per stream-chunk = 26
SENT = 1024.0               # separator value (bf16-exact, > any id)
INIT1 = -1024.0             # forward-scan initial
INITR = -896.0              # reverse-scan initial
DT = mybir.dt.bfloat16
F32 = mybir.dt.float32
Alu = mybir.AluOpType
Act = mybir.ActivationFunctionType

NSTREAM = 5                 # m, sgn, rsgn, e, me

_cache = {}


def build():
    nc = bass.Bass()
    x = nc.declare_dram_parameter("x", [NCH * P, FC], DT, isOutput=False)
    out = nc.declare_dram_parameter("out", [RPC, 4], F32, isOutput=True)
    out_re = out.rearrange("(p n) k -> p (n k)", p=P)   # [128, NT*4]

    with TileContext(nc) as tc:
        with (
            tc.tile_pool(name="persist", bufs=1) as pp,
            tc.tile_pool(name="work", bufs=2) as wp,
            tc.psum_pool(name="ps", bufs=1) as psp,
        ):
            # per-cell partial sums are <= 26, bf16-exact; split by writing
            # engine so each final reduce waits on a single semaphore
            stageD = pp.tile([P, 3 * NCH * CSEG * W], DT)
            stageA = pp.tile([P, 2 * NCH * CSEG * W], DT)
            stats = pp.tile([P, NSTREAM * NT], F32)
            feat = pp.tile([P, NT * 4], F32)

            # identity (stationary weights) built on-chip so matmuls depend
            # on DVE sems only (walrus MM struct allows a single sync wait)
            iota_row = pp.tile([P, P], mybir.dt.int32)
            iota_col = pp.tile([P, 1], F32)
            identt = pp.tile([P, P], DT)
            nc.gpsimd.iota(iota_row, [[1, P]], channel_multiplier=0)
            nc.gpsimd.iota(iota_col, [[0, 1]], channel_multiplier=1,
                           allow_small_or_imprecise_dtypes=True)
            nc.vector.tensor_scalar(identt, iota_row, iota_col, None, Alu.is_equal)
            negident = pp.tile([P, P], DT)
            nc.vector.tensor_scalar(negident, identt, -1.0, None, Alu.mult)
            warmps = psp.tile([P, 16], F32)
            # warm-up matmul: advances PE's DVE clock past identt/negident so
            # later matmuls need no extra identity wait
            nc.tensor.matmul(out=warmps, lhsT=identt, rhs=negident[:, 0:16],
                             start=True, stop=True)

            # one xc buffer per chunk, all DMAs issued up front: no WAR/WAW
            # waits, and the DMA engine runs well ahead of compute
            xcs = []
            for c in range(NCH):
                xc = wp.tile([P, FC], DT, tag="xc", bufs=NCH)
                nc.gpsimd.dma_start(out=xc, in_=x[c * P:(c + 1) * P, :])
                xcs.append(xc)

            for c in range(NCH):
                xc = xcs[c]
                m = wp.tile([P, FC], DT, tag="m", bufs=4)
                v1buf = wp.tile([P, FC + 2], DT, tag="v1buf")
                f1buf = wp.tile([P, FC + 2], DT, tag="f1buf")
                sgn = wp.tile([P, FC], DT, tag="sgn")
                rsgn = wp.tile([P, FC], DT, tag="rsgn")
                me = wp.tile([P, FC], DT, tag="me")

                # m = [x == 0] on DVE (keeps xc single-engine-read so the next
                # chunk's DMA WAR carries one semaphore; scans then wait on DVE
                # program order only)
                nc.vector.tensor_scalar(m, xc, 0.5, None, Alu.is_lt)
                nc.vector.memset(v1buf[:, 0:1], INIT1)
                nc.vector.memset(f1buf[:, FC + 1:FC + 2], INITR)

                # v1buf[1+t] = last nonzero at-or-before t  (v1[t] = v1buf[t])
                nc.vector.tensor_tensor_scan(
                    out=v1buf[:, 1:FC + 1], data0=m, data1=xc,
                    initial=INIT1, op0=Alu.mult, op1=Alu.add,
                )
                v1 = v1buf[:, 0:FC]
                # f1buf[1+t] = next nonzero at-or-after t (f1[t] = f1buf[t+2])
                nc.vector.tensor_tensor_scan(
                    out=f1buf[:, 1:FC + 1][:, ::-1], data0=m[:, ::-1],
                    data1=xc[:, ::-1],
                    initial=INITR, op0=Alu.mult, op1=Alu.add,
                )
                f1 = f1buf[:, 2:FC + 2]

                # PE toucher: one tiny matmul waiting on this chunk's DMA so
                # the d1 matmuls below carry a single (WAR) wait each
                nc.tensor.matmul(out=warmps, lhsT=identt, rhs=xc[:, 0:16],
                                 start=True, stop=True)
                # d1 = x - v1 computed on the PE: psum piece = I@x + (-I)@v1;
                # ScalarE reads Sign straight from PSUM
                NP_PIECE = 416
                for pi in range(FC // NP_PIECE):
                    lo, hi = pi * NP_PIECE, (pi + 1) * NP_PIECE
                    psd = psp.tile([P, NP_PIECE], F32, tag="psd", bufs=3,
                                   name=f"psd_{c}_{pi}")
                    nc.tensor.matmul(out=psd, lhsT=identt, rhs=xc[:, lo:hi],
                                     start=True, stop=False)
                    nc.tensor.matmul(out=psd, lhsT=negident, rhs=v1[:, lo:hi],
                                     start=False, stop=True)
                    nc.scalar.activation(sgn[:, lo:hi], psd, Act.Sign)
                nc.scalar.activation(rsgn, sgn, Act.Relu)
                # e in place over f1 (same-index elementwise; v1 must stay
                # intact for the PE's d1 matmuls)
                e = f1
                nc.vector.tensor_tensor(e, v1, f1, Alu.is_equal)
                nc.vector.tensor_tensor(me, m, e, Alu.mult)

                # per-segment sums via identity-matmul PSUM accumulation.
                # e/me/m are DVE-written: fold W and land in stats via a DVE
                # tensor_reduce straight from PSUM (single DVE-sem waits all
                # around). sgn/rsgn are ACT-written: ACT copies their psums to
                # stage2 so their matmuls' WAR stays on the ACT sem.
                for s_idx, s in enumerate((e, me, m, sgn, rsgn)):
                    if s_idx < 3:
                        ps = psp.tile([P, CSEG * W], F32, tag="ps", bufs=2)
                    else:
                        ps = psp.tile([P, CSEG * W], F32, tag="ps2", bufs=2)
                    s3 = s.rearrange("p (n w) -> p n w", w=SEG_W)
                    for cc in range(NMM):
                        nc.tensor.matmul(
                            out=ps, lhsT=identt,
                            rhs=s3[:, :, W * cc:W * cc + W],
                            start=(cc == 0), stop=(cc == NMM - 1),
                        )
                    if s_idx < 3:
                        dst = stageD[:, (s_idx * NCH + c) * CSEG * W:
                                     (s_idx * NCH + c + 1) * CSEG * W]
                        nc.vector.tensor_copy(dst, ps)
                    else:
                        dst = stageA[:, ((s_idx - 3) * NCH + c) * CSEG * W:
                                     ((s_idx - 3) * NCH + c + 1) * CSEG * W]
                        nc.scalar.copy(dst, ps)  # ACT streams stay on ACT sem

            stD = stageD.rearrange("p (q w) -> p q w", w=W)
            nc.vector.tensor_reduce(stats[:, 0:3 * NT], stD,
                                    mybir.AxisListType.X, Alu.add)
            stA = stageA.rearrange("p (q w) -> p q w", w=W)
            nc.vector.tensor_reduce(stats[:, 3 * NT:5 * NT], stA,
                                    mybir.AxisListType.X, Alu.add)

            def st(k):
                return stats[:, k * NT:(k + 1) * NT]

            E, ME, zc, D, pos = st(0), st(1), st(2), st(3), st(4)

            ep = pp
            n = ep.tile([P, NT], F32)
            nc.vector.tensor_scalar(n, zc, -1.0, float(L), Alu.mult, Alu.add)
            # rep = 257 - 2*pos + D ; inc = pos - 1 ; dec = pos - D - zc - 1
            inc = ep.tile([P, NT], F32)
            nc.vector.tensor_scalar(inc, pos, -1.0, None, Alu.add)
            t257 = ep.tile([P, NT], F32)
            nc.vector.tensor_scalar(t257, pos, -2.0, 257.0, Alu.mult, Alu.add)
            rep = ep.tile([P, NT], F32)
            nc.vector.tensor_tensor(rep, t257, D, Alu.add)
            u1 = ep.tile([P, NT], F32)
            nc.vector.tensor_tensor(u1, pos, D, Alu.subtract)
            u2 = ep.tile([P, NT], F32)
            nc.vector.tensor_tensor(u2, u1, zc, Alu.subtract)
            dec = ep.tile([P, NT], F32)
            nc.vector.tensor_scalar(dec, u2, -1.0, None, Alu.add)
            em2 = ep.tile([P, NT], F32)
            nc.vector.tensor_scalar(em2, E, -2.0, None, Alu.add)
            per = ep.tile([P, NT], F32)
            nc.vector.tensor_tensor(per, em2, ME, Alu.subtract)

            d1v = ep.tile([P, NT], F32)
            nc.vector.tensor_scalar(d1v, n, -1.0, 1.0, Alu.add, Alu.max)
            r1 = ep.tile([P, NT], F32)
            nc.vector.reciprocal(r1, d1v)
            d2v = ep.tile([P, NT], F32)
            nc.vector.tensor_scalar(d2v, n, -2.0, 1.0, Alu.add, Alu.max)
            r2 = ep.tile([P, NT], F32)
            nc.vector.reciprocal(r2, d2v)
            mask2 = ep.tile([P, NT], F32)
            nc.vector.tensor_scalar(mask2, n, 1.5, None, Alu.is_gt)
            mask4 = ep.tile([P, NT], F32)
            nc.vector.tensor_scalar(mask4, n, 3.5, None, Alu.is_gt)
            r1m = ep.tile([P, NT], F32)
            nc.vector.tensor_tensor(r1m, r1, mask2, Alu.mult)
            r2m = ep.tile([P, NT], F32)
            nc.vector.tensor_tensor(r2m, r2, mask4, Alu.mult)

            feat3 = feat.rearrange("p (n k) -> p n k", k=4)
            nc.vector.tensor_tensor(feat3[:, :, 0:1], rep, r1m, Alu.mult)
            nc.vector.tensor_tensor(feat3[:, :, 1:2], inc, r1m, Alu.mult)
            nc.vector.tensor_tensor(feat3[:, :, 2:3], dec, r1m, Alu.mult)
            nc.vector.tensor_tensor(feat3[:, :, 3:4], per, r2m, Alu.mult)

            # HWDGE (sync-engine) DMA: its completion sem is separate from the
            # 8-deep SWDGE sem pool, so no sem-reuse wait on input DMA 0
            nc.sync.dma_start(out=out_re, in_=feat)

    return nc


def _prep_core(x_core):
    """[RPC, 256] int32 -> [NCH*P, FC] bf16: pad 4 SENT sep cols per row,
    then reorder rows p*NT+j so each chunk (16 segments x all partitions)
    is one contiguous [128, FC] DMA block."""
    xpad = np.empty((RPC, SEG_W), dtype=ml_dtypes.bfloat16)
    xpad[:, :NSEP] = ml_dtypes.bfloat16(SENT)
    xpad[:, NSEP:] = x_core.astype(ml_dtypes.bfloat16)
    arr = xpad.reshape(P, NCH, CSEG, SEG_W).transpose(1, 0, 2, 3)
    return np.ascontiguousarray(arr.reshape(NCH * P, FC))


def kernel(x: np.ndarray) -> np.ndarray:
    if "nc" not in _cache:
        _cache["nc"] = build()
    nc = _cache["nc"]
    x = np.ascontiguousarray(np.asarray(x, dtype=np.int32))
    shards = x.reshape(NCORES, RPC, L)
    in_maps = [{"x": _prep_core(shards[i])} for i in range(NCORES)]
    res = run_bass_kernel_spmd(nc, in_maps, list(range(NCORES)))
    outs = [res.results[i]["out"] for i in range(NCORES)]
    return np.concatenate(outs, axis=0).astype(np.float32)


# revision 31
# speedup vs baseline: 1.3492x; 1.0179x over previous
import re
import sys

sys.path.insert(0, "/opt/trn_rl_repo")

import ml_dtypes
import numpy as np

from concourse import bass, mybir
from concourse.bass_utils import run_bass_kernel_spmd
from concourse.tile import TileContext
from concourse.vector_clock import ScopedClock, VectorClock


def _drain_and_barrier_split_waits(self, tick_clock, wait_clock):
    # Walrus codegen caps sync waits per CTRL instruction at 2; the stock
    # exit drain carries one wait per busy proc and fails to compile. Emit
    # one single-wait SP nop per proc ahead of the drain instead.
    ticks = [int(v) for v in re.findall(r"\d+", str(tick_clock.global_clock))]
    for p, t in enumerate(ticks):
        if t > 0:
            partial = VectorClock()
            partial.require_at_least(p, t)
            nop = self.nc.sync.nop(nofuse=True)
            wait_clock.add_sem_waits(nop.ins, ScopedClock({None: partial}))
    self.nc.sync.drain()
    self.nc.all_engine_barrier()
    assert self.sems is not None
    popped = self.nc._tile_sem_poison_stack.pop()
    assert popped is self._sem_poison
    self.nc.clear_and_free_semaphores(list(self.sems.allocated().values()))
    self.nc.all_engine_barrier()


TileContext._drain_and_barrier = _drain_and_barrier_split_waits

B, L = 131072, 256
NCORES = 8
RPC = B // NCORES           # rows per core = 16384
P = 128                     # partitions
NT = RPC // P               # segments per partition = 128
NSEP = 4                    # separator columns per segment
SEG_W = L + NSEP            # 260
NCH = 8                     # chunks per core
CSEG = NT // NCH            # segments per chunk = 16
FC = CSEG * SEG_W           # free elems per chunk = 4160
W = 10                      # positions folded per psum column (260 = 26*10)
NMM = SEG_W // W            # matmuls per stream-chunk = 26
SENT = 1024.0               # separator value (bf16-exact, > any id)
INIT1 = -1024.0             # forward-scan initial
INITR = -896.0              # reverse-scan initial
DT = mybir.dt.bfloat16
F32 = mybir.dt.float32
Alu = mybir.AluOpType
Act = mybir.ActivationFunctionType

NSTREAM = 5                 # m, sgn, rsgn, e, me

_cache = {}


def build():
    nc = bass.Bass()
    x = nc.declare_dram_parameter("x", [NCH * P, FC], DT, isOutput=False)
    out = nc.declare_dram_parameter("out", [RPC, 4], F32, isOutput=True)
    out_re = out.rearrange("(p n) k -> p (n k)", p=P)   # [128, NT*4]

    with TileContext(nc) as tc:
        with (
            tc.tile_pool(name="persist", bufs=1) as pp,
            tc.tile_pool(name="work", bufs=2) as wp,
            tc.psum_pool(name="ps", bufs=1) as psp,
        ):
            # per-cell partial sums are <= 26, bf16-exact; split by writing
            # engine so each final reduce waits on a single semaphore
            stageD = pp.tile([P, 3 * NCH * CSEG * W], DT)
            stageA = pp.tile([P, 2 * NCH * CSEG * W], DT)
            stats = pp.tile([P, NSTREAM * NT], F32)
            feat = pp.tile([P, NT * 4], F32)

            # identity (stationary weights) built on-chip so matmuls depend
            # on DVE sems only (walrus MM struct allows a single sync wait)
            iota_row = pp.tile([P, P], mybir.dt.int32)
            iota_col = pp.tile([P, 1], F32)
            identt = pp.tile([P, P], DT)
            nc.gpsimd.iota(iota_row, [[1, P]], channel_multiplier=0)
            nc.gpsimd.iota(iota_col, [[0, 1]], channel_multiplier=1,
                           allow_small_or_imprecise_dtypes=True)
            nc.vector.tensor_scalar(identt, iota_row, iota_col, None, Alu.is_equal)
            negident = pp.tile([P, P], DT)
            nc.vector.tensor_scalar(negident, identt, -1.0, None, Alu.mult)
            warmps = psp.tile([P, 16], F32)
            # warm-up matmul: advances PE's DVE clock past identt/negident so
            # later matmuls need no extra identity wait
            nc.tensor.matmul(out=warmps, lhsT=identt, rhs=negident[:, 0:16],
                             start=True, stop=True)

            # one xc buffer per chunk, all DMAs issued up front: no WAR/WAW
            # waits, and the DMA engine runs well ahead of compute
            xcs = []
            for c in range(NCH):
                xc = wp.tile([P, FC], DT, tag="xc", bufs=NCH)
                nc.gpsimd.dma_start(out=xc, in_=x[c * P:(c + 1) * P, :])
                xcs.append(xc)

            for c in range(NCH):
                xc = xcs[c]
                m = wp.tile([P, FC], DT, tag="m", bufs=4)
                v1buf = wp.tile([P, FC + 2], DT, tag="v1buf")
                f1buf = wp.tile([P, FC + 2], DT, tag="f1buf")
                sgn = wp.tile([P, FC], DT, tag="sgn")
                rsgn = wp.tile([P, FC], DT, tag="rsgn")
                me = wp.tile([P, FC], DT, tag="me")

                # m = [x == 0] on DVE (keeps xc single-engine-read so the next
                # chunk's DMA WAR carries one semaphore; scans then wait on DVE
                # program order only)
                nc.vector.tensor_scalar(m, xc, 0.5, None, Alu.is_lt)
                nc.vector.memset(v1buf[:, 0:1], INIT1)
                nc.vector.memset(f1buf[:, FC + 1:FC + 2], INITR)

                # v1buf[1+t] = last nonzero at-or-before t  (v1[t] = v1buf[t])
                nc.vector.tensor_tensor_scan(
                    out=v1buf[:, 1:FC + 1], data0=m, data1=xc,
                    initial=INIT1, op0=Alu.mult, op1=Alu.add,
                )
                v1 = v1buf[:, 0:FC]
                # f1buf[1+t] = next nonzero at-or-after t (f1[t] = f1buf[t+2])
                nc.vector.tensor_tensor_scan(
                    out=f1buf[:, 1:FC + 1][:, ::-1], data0=m[:, ::-1],
                    data1=xc[:, ::-1],
                    initial=INITR, op0=Alu.mult, op1=Alu.add,
                )
                f1 = f1buf[:, 2:FC + 2]

                # PE toucher: one tiny matmul waiting on this chunk's DMA so
                # the d1 matmuls below carry a single (WAR) wait each
                nc.tensor.matmul(out=warmps, lhsT=identt, rhs=xc[:, 0:16],
                                 start=True, stop=True)
                # d1 = x - v1 computed on the PE: psum piece = I@x + (-I)@v1;
                # ScalarE reads Sign straight from PSUM
                NP_PIECE = 416
                for pi in range(FC // NP_PIECE):
                    lo, hi = pi * NP_PIECE, (pi + 1) * NP_PIECE
                    psd = psp.tile([P, NP_PIECE], F32, tag="psd", bufs=3,
                                   name=f"psd_{c}_{pi}")
                    nc.tensor.matmul(out=psd, lhsT=identt, rhs=xc[:, lo:hi],
                                     start=True, stop=False)
                    nc.tensor.matmul(out=psd, lhsT=negident, rhs=v1[:, lo:hi],
                                     start=False, stop=True)
                    nc.scalar.activation(sgn[:, lo:hi], psd, Act.Sign)
                nc.scalar.activation(rsgn, sgn, Act.Relu)
                # e in place over f1 (same-index elementwise; v1 must stay
                # intact for the PE's d1 matmuls)
                e = f1
                nc.vector.tensor_tensor(e, v1, f1, Alu.is_equal)
                nc.vector.tensor_tensor(me, m, e, Alu.mult)

                # per-segment sums via identity-matmul PSUM accumulation.
                # e/me/m are DVE-written: fold W and land in stats via a DVE
                # tensor_reduce straight from PSUM (single DVE-sem waits all
                # around). sgn/rsgn are ACT-written: ACT copies their psums to
                # stage2 so their matmuls' WAR stays on the ACT sem.
                for s_idx, s in enumerate((e, me, m, sgn, rsgn)):
                    if s_idx < 3:
                        ps = psp.tile([P, CSEG * W], F32, tag="ps", bufs=2)
                    else:
                        ps = psp.tile([P, CSEG * W], F32, tag="ps2", bufs=2)
                    s3 = s.rearrange("p (n w) -> p n w", w=SEG_W)
                    for cc in range(NMM):
                        nc.tensor.matmul(
                            out=ps, lhsT=identt,
                            rhs=s3[:, :, W * cc:W * cc + W],
                            start=(cc == 0), stop=(cc == NMM - 1),
                        )
                    if s_idx < 3:
                        dst = stageD[:, (s_idx * NCH + c) * CSEG * W:
                                     (s_idx * NCH + c + 1) * CSEG * W]
                        nc.vector.tensor_copy(dst, ps)
                    else:
                        dst = stageA[:, ((s_idx - 3) * NCH + c) * CSEG * W:
                                     ((s_idx - 3) * NCH + c + 1) * CSEG * W]
                        nc.scalar.copy(dst, ps)  # ACT streams stay on ACT sem

            stD4 = stageD.rearrange("p (s c q) -> p s c q", s=3, c=NCH)
            stA4 = stageA.rearrange("p (s c q) -> p s c q", s=2, c=NCH)
            std4 = stats.rearrange("p (s c n) -> p s c n", s=NSTREAM, c=NCH)
            for lo, hi in ((0, NCH // 2), (NCH // 2, NCH)):
                nc.vector.tensor_reduce(
                    std4[:, 0:3, lo:hi, :],
                    stD4[:, :, lo:hi, :].rearrange("p s c (n w) -> p s c n w", w=W),
                    mybir.AxisListType.X, Alu.add)
                nc.vector.tensor_reduce(
                    std4[:, 3:5, lo:hi, :],
                    stA4[:, :, lo:hi, :].rearrange("p s c (n w) -> p s c n w", w=W),
                    mybir.AxisListType.X, Alu.add)

            def st(k):
                return stats[:, k * NT:(k + 1) * NT]

            E, ME, zc, D, pos = st(0), st(1), st(2), st(3), st(4)

            ep = pp
            n = ep.tile([P, NT], F32)
            nc.vector.tensor_scalar(n, zc, -1.0, float(L), Alu.mult, Alu.add)
            # rep = 257 - 2*pos + D ; inc = pos - 1 ; dec = pos - D - zc - 1
            inc = ep.tile([P, NT], F32)
            nc.vector.tensor_scalar(inc, pos, -1.0, None, Alu.add)
            t257 = ep.tile([P, NT], F32)
            nc.vector.tensor_scalar(t257, pos, -2.0, 257.0, Alu.mult, Alu.add)
            rep = ep.tile([P, NT], F32)
            nc.vector.tensor_tensor(rep, t257, D, Alu.add)
            u1 = ep.tile([P, NT], F32)
            nc.vector.tensor_tensor(u1, pos, D, Alu.subtract)
            u2 = ep.tile([P, NT], F32)
            nc.vector.tensor_tensor(u2, u1, zc, Alu.subtract)
            dec = ep.tile([P, NT], F32)
            nc.vector.tensor_scalar(dec, u2, -1.0, None, Alu.add)
            em2 = ep.tile([P, NT], F32)
            nc.vector.tensor_scalar(em2, E, -2.0, None, Alu.add)
            per = ep.tile([P, NT], F32)
            nc.vector.tensor_tensor(per, em2, ME, Alu.subtract)

            d1v = ep.tile([P, NT], F32)
            nc.vector.tensor_scalar(d1v, n, -1.0, 1.0, Alu.add, Alu.max)
            r1 = ep.tile([P, NT], F32)
            nc.vector.reciprocal(r1, d1v)
            d2v = ep.tile([P, NT], F32)
            nc.vector.tensor_scalar(d2v, n, -2.0, 1.0, Alu.add, Alu.max)
            r2 = ep.tile([P, NT], F32)
            nc.vector.reciprocal(r2, d2v)
            mask2 = ep.tile([P, NT], F32)
            nc.vector.tensor_scalar(mask2, n, 1.5, None, Alu.is_gt)
            mask4 = ep.tile([P, NT], F32)
            nc.vector.tensor_scalar(mask4, n, 3.5, None, Alu.is_gt)
            r1m = ep.tile([P, NT], F32)
            nc.vector.tensor_tensor(r1m, r1, mask2, Alu.mult)
            r2m = ep.tile([P, NT], F32)
            nc.vector.tensor_tensor(r2m, r2, mask4, Alu.mult)

            feat3 = feat.rearrange("p (n k) -> p n k", k=4)
            nc.vector.tensor_tensor(feat3[:, :, 0:1], rep, r1m, Alu.mult)
            nc.vector.tensor_tensor(feat3[:, :, 1:2], inc, r1m, Alu.mult)
            nc.vector.tensor_tensor(feat3[:, :, 2:3], dec, r1m, Alu.mult)
            nc.vector.tensor_tensor(feat3[:, :, 3:4], per, r2m, Alu.mult)

            # HWDGE (sync-engine) DMA: its completion sem is separate from the
            # 8-deep SWDGE sem pool, so no sem-reuse wait on input DMA 0
            nc.sync.dma_start(out=out_re, in_=feat)

    return nc


def _prep_core(x_core):
    """[RPC, 256] int32 -> [NCH*P, FC] bf16: pad 4 SENT sep cols per row,
    then reorder rows p*NT+j so each chunk (16 segments x all partitions)
    is one contiguous [128, FC] DMA block."""
    xpad = np.empty((RPC, SEG_W), dtype=ml_dtypes.bfloat16)
    xpad[:, :NSEP] = ml_dtypes.bfloat16(SENT)
    xpad[:, NSEP:] = x_core.astype(ml_dtypes.bfloat16)
    arr = xpad.reshape(P, NCH, CSEG, SEG_W).transpose(1, 0, 2, 3)
    return np.ascontiguousarray(arr.reshape(NCH * P, FC))


def kernel(x: np.ndarray) -> np.ndarray:
    if "nc" not in _cache:
        _cache["nc"] = build()
    nc = _cache["nc"]
    x = np.ascontiguousarray(np.asarray(x, dtype=np.int32))
    shards = x.reshape(NCORES, RPC, L)
    in_maps = [{"x": _prep_core(shards[i])} for i in range(NCORES)]
    res = run_bass_kernel_spmd(nc, in_maps, list(range(NCORES)))
    outs = [res.results[i]["out"] for i in range(NCORES)]
    return np.concatenate(outs, axis=0).astype(np.float32)
